# revision 1
# baseline (speedup 1.0000x reference)
"""EquivSetGNN forward on 8 Trainium2 NeuronCores (Bass/Tile).

Sharding: nodes (and their incident nnz entries, src-partitioned) are split
evenly across 8 cores. Per layer:
  V->E: each core gathers h[src] rows for its (dst-sorted) entries via
        dma_gather, segment-sums them per 128-wide dst window using PE
        matmuls with DVE-built one-hot selection matrices (PSUM
        accumulation), scales by 1/deg(dst) during flush, producing a
        partial Xe table; AllReduce across cores yields the full Xe.
  E->V: gathers Xe[dst] rows for its (src-window-ordered) entries, same
        one-hot matmul segment-sum into node windows scaled by 1/deg(src).
Dense MLP phases run in feature-transposed layout [64, n] per core.
Readout: per-graph sums via one-hot matmuls + small AllReduce + 1/count.
"""
import sys

sys.path.insert(0, "/opt/trn_rl_repo")

import ml_dtypes
import numpy as np

import concourse.bass as bass
import concourse.bacc as bacc
import concourse.mybir as mybir
import concourse.tile as tile
from concourse.bass_utils import run_bass_kernel_spmd
from concourse.library_config import mlp as mlp_lib
from concourse.masks import make_identity
from contextlib import ExitStack

F32 = mybir.dt.float32
BF16 = mybir.dt.bfloat16
I16 = mybir.dt.int16
AF = mybir.ActivationFunctionType
ALU = mybir.AluOpType


class Cfg:
    def __init__(self, N=100000, E=50000, FT=128, HID=64, CLS_H=64, NCLS=32,
                 NGRAPH=256, NLAYER=2, NCORES=8, EHALF=32768):
        self.N, self.E, self.FT, self.HID = N, E, FT, HID
        self.CLS_H, self.NCLS, self.NGRAPH, self.NLAYER = CLS_H, NCLS, NGRAPH, NLAYER
        self.NCORES = NCORES
        self.EHALF = EHALF  # int16 split point for gathers from Xe
        assert N % NCORES == 0
        self.NLOC = N // NCORES
        self.NW = -(-self.NLOC // 128)       # node windows per core
        self.EW = -(-E // 128)               # edge windows
        self.GW = -(-NGRAPH // 128)          # graph windows
        self.EPAD = self.EW * 128


def _ceil(a, b):
    return -(-a // b)


def _wrap16(idx):
    """flat idx array -> [128, L/16] int16 wrapped layout (replicated x8)."""
    a = np.asarray(idx, np.int16).reshape(-1, 16).T
    return np.ascontiguousarray(np.tile(a, (8, 1)))


def _gath_layout(vals, fill, dtype):
    """flat [L] -> [128, L/128] gathered layout (entry k at [k%128, k//128])."""
    L = len(vals)
    assert L % 128 == 0
    return np.ascontiguousarray(np.asarray(vals, dtype).reshape(L // 128, 128).T)


def prep(cfg, X, v2e_src, v2e_dst, all_batch):
    """Host preprocessing -> (shared_structure, per_core_input_maps)."""
    c = cfg
    src = np.asarray(v2e_src, np.int64)
    dst = np.asarray(v2e_dst, np.int64)
    batch = np.asarray(all_batch, np.int64)

    d_deg = np.bincount(dst, minlength=c.E).astype(np.float32)
    c_deg = np.bincount(src, minlength=c.N).astype(np.float32)
    recip_d = (1.0 / np.maximum(d_deg, 1.0)).astype(np.float32)
    recip_c = (1.0 / np.maximum(c_deg, 1.0)).astype(np.float32)

    cores = []
    for ci in range(c.NCORES):
        lo, hi = np.searchsorted(src, [c.NLOC * ci, c.NLOC * (ci + 1)])
        s = (src[lo:hi] - c.NLOC * ci).astype(np.int64)
        e = dst[lo:hi]
        cores.append((s, e))

    # ---- V->E stream (dst-sorted), per edge-window block counts ----
    cntA = np.zeros((c.NCORES, c.EW), np.int64)
    coreA = []
    for ci, (s, e) in enumerate(cores):
        order = np.argsort(e, kind="stable")
        sA, eA = s[order], e[order]
        win = eA >> 7
        cntA[ci] = np.bincount(win, minlength=c.EW)
        coreA.append((sA, eA, win))
    BA = _ceil(cntA.max(axis=0), 1)
    BA = -(-BA // 128)  # blocks per window (shared)
    BA = np.maximum(BA, 0)
    capA = BA * 128
    offA = np.concatenate([[0], np.cumsum(capA)])
    LA = int(offA[-1])
    nblkA = LA // 128

    # ---- E->V stream (node-window ordered, L then H per window) ----
    cntL = np.zeros((c.NCORES, c.NW), np.int64)
    cntH = np.zeros((c.NCORES, c.NW), np.int64)
    coreB = []
    for ci, (s, e) in enumerate(cores):
        m = s >> 7
        isH = (e >= c.EHALF).astype(np.int64)
        order = np.argsort(m * 2 + isH, kind="stable")
        sB, eB, mB, hB = s[order], e[order], m[order], isH[order]
        cntL[ci] = np.bincount(mB[hB == 0], minlength=c.NW)
        cntH[ci] = np.bincount(mB[hB == 1], minlength=c.NW)
        coreB.append((sB, eB, mB, hB))
    BL = -(-cntL.max(axis=0) // 128)
    BH = -(-cntH.max(axis=0) // 128)
    capL, capH = BL * 128, BH * 128
    # L parts of all windows first, then all H parts
    offL = np.concatenate([[0], np.cumsum(capL)])[:-1]
    LBL = int(capL.sum())
    offH = LBL + np.concatenate([[0], np.cumsum(capH)])[:-1]
    LB = LBL + int(capH.sum())
    nblkB = LB // 128
    nblkBL = LBL // 128

    shared = dict(BA=BA, BL=BL, BH=BH, LA=LA, LB=LB, nblkA=nblkA, nblkB=nblkB,
                  nblkBL=nblkBL, offA=offA, offL=offL, offH=offH)

    # graph counts
    gcnt = np.bincount(batch, minlength=c.NGRAPH).astype(np.float32)
    recip_g = (1.0 / np.maximum(gcnt, 1.0)).astype(np.float32)
    recip_g_win = np.zeros((128, c.GW), np.float32)
    for g in range(c.NGRAPH):
        recip_g_win[g % 128, g // 128] = recip_g[g]

    in_maps = []
    for ci in range(c.NCORES):
        # V->E placement
        sA, eA, winA = coreA[ci]
        startsA = np.searchsorted(winA, np.arange(c.EW))
        place = offA[winA] + (np.arange(len(winA)) - startsA[winA])
        gidxA = np.zeros(LA, np.int64)
        idsA = np.full(LA, -1.0, np.float32)
        gidxA[place] = sA
        idsA[place] = (eA - (winA << 7)).astype(np.float32)

        # E->V placement
        sB, eB, mB, hB = coreB[ci]
        keyB = mB * 2 + hB
        startsB = np.searchsorted(keyB, np.arange(2 * c.NW))
        base = np.where(hB == 0, offL[mB], offH[mB])
        place = base + (np.arange(len(keyB)) - startsB[keyB])
        gidxB = np.zeros(LB, np.int64)
        idsB = np.full(LB, -1.0, np.float32)
        gidxB[place] = np.where(hB == 0, eB, eB - c.EHALF)
        idsB[place] = (sB - (mB << 7)).astype(np.float32)

        # per-node arrays in window layout [128, NW]
        npad = c.NW * 128
        cw = np.zeros(npad, np.float32)
        cw[:c.NLOC] = recip_c[c.NLOC * ci: c.NLOC * (ci + 1)]
        recip_c_win = np.ascontiguousarray(cw.reshape(c.NW, 128).T)
        mw = np.zeros(npad, np.float32)
        mw[:c.NLOC] = (c_deg[c.NLOC * ci: c.NLOC * (ci + 1)] > 0).astype(np.float32)
        mask_win = np.ascontiguousarray(mw.reshape(c.NW, 128).T)
        bw = np.full(npad, -1.0, np.float32)
        bw[:c.NLOC] = batch[c.NLOC * ci: c.NLOC * (ci + 1)].astype(np.float32)
        ids_g = np.ascontiguousarray(bw.reshape(c.NW, 128).T)
        bw1 = np.where(bw < 0, -1.0, bw - 128.0).astype(np.float32)
        ids_g1 = np.ascontiguousarray(bw1.reshape(c.NW, 128).T)

        dw = np.zeros(c.EW * 128, np.float32)
        dw[:c.E] = recip_d
        recip_d_win = np.ascontiguousarray(dw.reshape(c.EW, 128).T)

        iota2d = np.broadcast_to(np.arange(128).astype(ml_dtypes.bfloat16), (128, 128))
        iota2d = np.ascontiguousarray(iota2d.reshape(128, 1, 128))

        m = {
            "Xs": np.ascontiguousarray(X[c.NLOC * ci: c.NLOC * (ci + 1)]).astype(np.float32),
            "gidxA": _wrap16(gidxA), "idsA": _gath_layout(idsA, -1.0, ml_dtypes.bfloat16),
            "gidxB": _wrap16(gidxB), "idsB": _gath_layout(idsB, -1.0, ml_dtypes.bfloat16),
            "recip_c_win": recip_c_win, "mask_win": mask_win,
            "recip_d_win": recip_d_win, "ids_g": ids_g, "ids_g1": ids_g1,
            "recip_g_win": recip_g_win, "iota2d": iota2d,
        }
        in_maps.append(m)
    return shared, in_maps


def build(cfg, sh, weights_shapes, debug_taps=False):
    """Build the SPMD Bass program. weights_shapes: dict name->shape."""
    c = cfg
    nc = bacc.Bacc("TRN2", debug=False, num_swdge_queues=1)
    HID = c.HID

    # ---------- I/O ----------
    Xs = nc.declare_dram_parameter("Xs", [c.NLOC, c.FT], F32, isOutput=False)
    gidxA_d = nc.declare_dram_parameter("gidxA", [128, sh["LA"] // 16], I16, isOutput=False)
    idsA_d = nc.declare_dram_parameter("idsA", [128, sh["nblkA"]], BF16, isOutput=False)
    gidxB_d = nc.declare_dram_parameter("gidxB", [128, sh["LB"] // 16], I16, isOutput=False)
    idsB_d = nc.declare_dram_parameter("idsB", [128, sh["nblkB"]], BF16, isOutput=False)
    recip_c_d = nc.declare_dram_parameter("recip_c_win", [128, c.NW], F32, isOutput=False)
    mask_d = nc.declare_dram_parameter("mask_win", [128, c.NW], F32, isOutput=False)
    recip_d_d = nc.declare_dram_parameter("recip_d_win", [128, c.EW], F32, isOutput=False)
    ids_g_d = nc.declare_dram_parameter("ids_g", [128, c.NW], F32, isOutput=False)
    ids_g1_d = nc.declare_dram_parameter("ids_g1", [128, c.NW], F32, isOutput=False)
    recip_g_d = nc.declare_dram_parameter("recip_g_win", [128, c.GW], F32, isOutput=False)
    iota_d = nc.declare_dram_parameter("iota2d", [128, 1, 128], BF16, isOutput=False)
    wparams = {}
    for name, shp in weights_shapes.items():
        wparams[name] = nc.declare_dram_parameter(name, list(shp), F32, isOutput=False)
    out_d = nc.declare_dram_parameter("out", [c.NGRAPH, c.NCLS], F32, isOutput=True)
    taps = {}
    if debug_taps:
        taps["h"] = nc.declare_dram_parameter("dbg_h", [c.NLOC, c.HID], F32, isOutput=True)
        taps["xe"] = nc.declare_dram_parameter("dbg_xe", [c.EPAD, c.HID], F32, isOutput=True)
        taps["spart"] = nc.declare_dram_parameter("dbg_spart", [c.EPAD, c.HID], F32, isOutput=True)
        taps["xT"] = nc.declare_dram_parameter("dbg_xT", [c.HID, c.NLOC], F32, isOutput=True)
        taps["yT"] = nc.declare_dram_parameter("dbg_yT", [c.HID, c.NLOC], F32, isOutput=True)

    # ---------- internal DRAM ----------
    h_dram = nc.dram_tensor("h_tab", [c.NLOC, HID], F32)
    EW_HALF = 0  # single AR (split not worth it per cost model)
    N_CC = 2 if EW_HALF > 0 else 1
    if EW_HALF > 0:
        s_part0 = nc.dram_tensor("s_part0", [EW_HALF * 128, HID], F32)
    s_part1 = nc.dram_tensor("s_part1", [c.EPAD - EW_HALF * 128, HID], F32)
    xe_dram = nc.dram_tensor("xe_tab", [c.EPAD, HID], F32, addr_space="Shared")
    xT_dram = nc.dram_tensor("xT", [HID, c.NLOC], F32)
    x0h_dram = nc.dram_tensor("x0h", [HID, c.NLOC], F32)
    yT_dram = nc.dram_tensor("yT", [HID, c.NLOC], F32)
    gsum_part = nc.dram_tensor("gsum_part", [c.GW * 128, c.NCLS], F32)
    gsum_full = nc.dram_tensor("gsum_full", [c.GW * 128, c.NCLS], F32, addr_space="Shared")

    rg = [list(range(c.NCORES))]

    with tile.TileContext(nc) as tc:
        ctx = ExitStack()
        const = ctx.enter_context(tc.tile_pool(name="const", bufs=1))
        sb = ctx.enter_context(tc.tile_pool(name="sb", bufs=2))
        gp = ctx.enter_context(tc.tile_pool(name="gp", bufs=4))
        ohp = ctx.enter_context(tc.tile_pool(name="ohp", bufs=2))
        ohgp = ctx.enter_context(tc.tile_pool(name="ohgp", bufs=2))
        flp = ctx.enter_context(tc.tile_pool(name="flp", bufs=3))
        ps_win = ctx.enter_context(tc.tile_pool(name="ps_win", bufs=2, space="PSUM"))
        ps_dense = ctx.enter_context(tc.tile_pool(name="ps_dense", bufs=2, space="PSUM"))
        ps_tr = ctx.enter_context(tc.tile_pool(name="ps_tr", bufs=1, space="PSUM"))
        ps_g = ctx.enter_context(tc.tile_pool(name="ps_g", bufs=1, space="PSUM"))

        # ---------- constants in SBUF ----------
        def load_const(dram, shape, dtype=F32):
            t = const.tile(shape, dtype, tag=f"c_{dram.name}")
            sl = tuple(slice(None) for _ in shape)
            nc.sync.dma_start(out=t[sl], in_=dram[sl])
            return t

        ident = const.tile([128, 128], F32)
        make_identity(nc, ident[:, :])
        iota = load_const(iota_d, [128, 1, 128], BF16)
        gidxA = load_const(gidxA_d, [128, sh["LA"] // 16], I16)
        idsA = load_const(idsA_d, [128, sh["nblkA"]], BF16)
        gidxB = load_const(gidxB_d, [128, sh["LB"] // 16], I16)
        idsB = load_const(idsB_d, [128, sh["nblkB"]], BF16)
        recip_c = load_const(recip_c_d, [128, c.NW])
        maskw = load_const(mask_d, [128, c.NW])
        recip_dw = load_const(recip_d_d, [128, c.EW])
        ids_g = load_const(ids_g_d, [128, c.NW])
        ids_g1 = load_const(ids_g1_d, [128, c.NW])
        recip_gw = load_const(recip_g_d, [128, c.GW])
        W = {k: load_const(v, list(v.shape)) for k, v in wparams.items()}
        # biases as [HID,1] column APs
        bias = {}
        for bn, dim in [("b_in", HID), ("b1a", HID), ("b1b", HID), ("b3", HID),
                        ("bc1", c.CLS_H)]:
            bias[bn] = W[bn]
        b2_rep = W["b2"]          # [128, HID], host-replicated
        bc2_rep = W["bc2"]        # [128, NCLS], host-replicated

        NWIN_LAST = c.NLOC - 128 * (c.NW - 1)  # rows in last node window


        def nodeblk(i):
            return slice(128 * i, min(128 * (i + 1), c.NLOC))

        def blkrows(i):
            return min(128 * (i + 1), c.NLOC) - 128 * i

        # ---------- input layer: x = relu(X @ W_in + b_in), transposed ----------
        for b in range(c.NW):
            r = blkrows(b)
            xblk = sb.tile([128, c.FT], F32, tag="xblk")
            nc.sync.dma_start(out=xblk[:r, :], in_=Xs[nodeblk(b), :])
            pt = ps_tr.tile([128, 128], F32, tag="ptr")
            nc.tensor.transpose(out=pt[:c.FT, :r], in_=xblk[:r, :c.FT], identity=ident[:r, :r])
            xTb = sb.tile([128, 128], F32, tag="xTb")
            nc.scalar.activation(out=xTb[:c.FT, :r], in_=pt[:c.FT, :r], func=AF.Copy)
            pd = ps_dense.tile([HID, 512], F32, tag="pd")
            nc.tensor.matmul(out=pd[:HID, :r], lhsT=W["W_in"][:, :], rhs=xTb[:c.FT, :r],
                             start=True, stop=True)
            xt = sb.tile([HID, 128], F32, tag="xt")
            nc.scalar.activation(out=xt[:, :r], in_=pd[:HID, :r], func=AF.Relu,
                                 bias=bias["b_in"][:, 0:1])
            nc.sync.dma_start(out=xT_dram[:, nodeblk(b)], in_=xt[:, :r])
            x0 = sb.tile([HID, 128], F32, tag="x0")
            nc.vector.tensor_scalar_mul(x0[:, :r], xt[:, :r], 0.5)
            nc.sync.dma_start(out=x0h_dram[:, nodeblk(b)], in_=x0[:, :r])

        CH = 512

        def dense_chunks():
            o = 0
            while o < c.NLOC:
                yield o, min(CH, c.NLOC - o)
                o += CH

        for layer in range(c.NLAYER):
            # ---------- h = relu(x@W1a+b1a)@W1b + b1b; write row-major table ----
            for o, n in dense_chunks():
                xt = sb.tile([HID, CH], F32, tag="xt2")
                nc.sync.dma_start(out=xt[:, :n], in_=xT_dram[:, o:o + n])
                pd = ps_dense.tile([HID, 512], F32, tag="pd")
                nc.tensor.matmul(out=pd[:HID, :n], lhsT=W["W1a"][:, :], rhs=xt[:, :n],
                                 start=True, stop=True)
                ut = sb.tile([HID, CH], F32, tag="ut")
                nc.scalar.activation(out=ut[:, :n], in_=pd[:HID, :n], func=AF.Relu,
                                     bias=bias["b1a"][:, 0:1])
                pd2 = ps_dense.tile([HID, 512], F32, tag="pd")
                nc.tensor.matmul(out=pd2[:HID, :n], lhsT=W["W1b"][:, :], rhs=ut[:, :n],
                                 start=True, stop=True)
                ht = sb.tile([HID, CH], F32, tag="ht")
                nc.vector.tensor_scalar(ht[:, :n], pd2[:HID, :n], W["b1b"][:, 0:1], None,
                                        ALU.add)
                # transpose to row-major h table
                nb = _ceil(n, 128)
                for j in range(nb):
                    r = min(128, n - 128 * j)
                    pt = ps_tr.tile([128, 128], F32, tag="ptr")
                    nc.tensor.transpose(out=pt[:r, :HID], in_=ht[:HID, 128 * j:128 * j + r],
                                        identity=ident[:HID, :HID])
                    hrm = flp.tile([128, HID], F32, tag="hrm")
                    nc.scalar.activation(out=hrm[:r, :], in_=pt[:r, :HID], func=AF.Copy)
                    nc.sync.dma_start(out=h_dram[o + 128 * j: o + 128 * j + r, :],
                                      in_=hrm[:r, :])

            # ---------- V->E: gather h[src], one-hot matmul into dst windows ----
            def make_stream(idx_tile, ids_tile, regions, dtag):
                """regions: list of (blk_start, blk_end, src_ap). Returns
                get(b) -> (g_tile, oh_tile, col) with lazy 8-block chunk
                gathers that never cross region boundaries."""
                cache = {}

                def get(b):
                    for r0, r1, src_ap in regions:
                        if r0 <= b < r1:
                            c0 = r0 + ((b - r0) // 8) * 8
                            key = c0
                            if key not in cache:
                                nb = min(8, r1 - c0)
                                gf = gp.tile([128, 8, HID], F32, tag="f" + dtag)
                                nidx = 128 * nb
                                nc.gpsimd.dma_gather(
                                    out_ap=gf[:, :nb, :], in_ap=src_ap,
                                    idxs_ap=idx_tile[:, 8 * c0: 8 * c0 + 8 * nb],
                                    num_idxs=nidx, num_idxs_reg=nidx, elem_size=HID,
                                )
                                g = gp.tile([128, 8, HID], BF16, tag=dtag)
                                nc.scalar.activation(out=g[:, :nb, :], in_=gf[:, :nb, :],
                                                     func=AF.Copy)
                                oh = ohp.tile([128, 8, 128], BF16, tag="oh" + dtag)
                                nc.vector.tensor_tensor(
                                    out=oh[:, :nb, :],
                                    in0=ids_tile[:, c0:c0 + nb].to_broadcast([128, nb, 128]),
                                    in1=iota[:, :, :].to_broadcast([128, nb, 128]),
                                    op=ALU.is_equal,
                                )
                                cache[key] = (g, oh)
                            g, oh = cache[key]
                            return g, oh, b - c0
                    raise AssertionError(b)
                return get

            offA = sh["offA"]
            BA = sh["BA"]
            getA = make_stream(gidxA, idsA, [(0, sh["nblkA"], h_dram[:, :])], "gA")
            cc_sem = nc.alloc_semaphore(f"cc{layer}")
            FB = 4  # windows per flush batch
            for w0 in range(0, c.EW, FB):
                wn = min(FB, c.EW - w0)
                sfl = flp.tile([128, FB, HID], F32, tag="sfl")
                for dw_ in range(wn):
                    w = w0 + dw_
                    nblk = int(BA[w])
                    if nblk == 0:
                        nc.vector.memset(sfl[:, dw_, :], 0.0)
                        continue
                    b0 = int(offA[w]) // 128
                    pw = ps_win.tile([128, HID], F32, tag="pw")
                    for i in range(nblk):
                        g, oh, col = getA(b0 + i)
                        nc.tensor.matmul(out=pw[:, :], lhsT=oh[:, col, :],
                                         rhs=g[:, col, :],
                                         start=(i == 0), stop=(i == nblk - 1))
                    nc.scalar.activation(
                        out=sfl[:, dw_, :], in_=pw[:, :], func=AF.Copy,
                        scale=recip_dw[:, w:w + 1])
                if w0 < EW_HALF:
                    tgt = s_part0[128 * w0:128 * (w0 + wn), :]
                else:
                    tgt = s_part1[128 * (w0 - EW_HALF):128 * (w0 - EW_HALF + wn), :]
                nc.sync.dma_start(
                    out=tgt.rearrange("(j p) c -> p j c", p=128),
                    in_=sfl[:, :wn, :])
                if w0 + wn == EW_HALF:
                    # first-half AllReduce overlaps the rest of V->E
                    with tc.tile_critical():
                        nc.gpsimd.collective_compute(
                            "AllReduce", ALU.add, replica_groups=rg,
                            ins=[s_part0.ap().opt()],
                            outs=[xe_dram[0:EW_HALF * 128, :].opt()],
                        ).then_inc(cc_sem, 1)

            # ---------- second-half AllReduce ----------
            with tc.tile_critical():
                nc.gpsimd.collective_compute(
                    "AllReduce", ALU.add, replica_groups=rg,
                    ins=[s_part1.ap().opt()],
                    outs=[xe_dram[EW_HALF * 128:c.EPAD, :].opt()],
                ).then_inc(cc_sem, 1)

            # Xe-independent dense term overlaps the AllReduce:
            # tb[m] = x@W2a + b2 per node window
            tbbuf = const.tile([128, c.NW, HID], F32, tag="tbbuf")
            for m in range(c.NW):
                if m % 4 == 0:
                    o4 = 128 * m
                    n4 = min(512, c.NLOC - o4)
                    xt4p = sb.tile([HID, 512], F32, tag="xt3")
                    nc.sync.dma_start(out=xt4p[:, :n4], in_=xT_dram[:, o4:o4 + n4])
                rows = blkrows(m)
                co = 128 * m - o4
                pdp = ps_tr.tile([128, 128], F32, tag="ptr")
                nc.tensor.matmul(out=pdp[:rows, :HID], lhsT=xt4p[:, co:co + rows],
                                 rhs=W["W2a"][:, :], start=True, stop=True)
                nc.vector.tensor_tensor(out=tbbuf[:rows, m, :], in0=pdp[:rows, :HID],
                                        in1=b2_rep[:rows, :], op=ALU.add)

            with tc.tile_critical():
                nc.gpsimd.wait_ge(cc_sem, N_CC)
            tc.strict_bb_all_engine_barrier()

            if debug_taps and layer == 0:
                nc.sync.dma_start(out=taps["h"][:, :], in_=h_dram[:, :])
                nc.sync.dma_start(out=taps["xe"][:, :], in_=xe_dram[:, :])
                nc.sync.dma_start(out=taps["spart"][:, :], in_=s_part[:, :])
            # ---------- E->V + node-window dense update ----------
            BL, BH = sh["BL"], sh["BH"]
            offL, offH = sh["offL"], sh["offH"]
            getB = make_stream(gidxB, idsB,
                               [(0, sh["nblkBL"], xe_dram[:, :]),
                                (sh["nblkBL"], sh["nblkB"], xe_dram[c.EHALF:, :])],
                               "gB")
            yt4w = {}
            for m in range(c.NW):
                if m % 4 == 0:
                    o4 = 128 * m
                    n4 = min(512, c.NLOC - o4)
                    yt4 = sb.tile([HID, 512], F32, tag="yt")
                    x04 = sb.tile([HID, 512], F32, tag="x0b")
                    nc.sync.dma_start(out=x04[:, :n4], in_=x0h_dram[:, o4:o4 + n4])
                    yt4w[m // 4] = (yt4, x04, o4, n4)
                rows = blkrows(m)
                pw = ps_win.tile([128, HID], F32, tag="pw")
                total = int(BL[m]) + int(BH[m])
                done = 0
                for nblk, off in ((int(BL[m]), int(offL[m])),
                                  (int(BH[m]), int(offH[m]))):
                    b0 = off // 128
                    for i in range(nblk):
                        g, oh, col = getB(b0 + i)
                        nc.tensor.matmul(out=pw[:, :], lhsT=oh[:, col, :],
                                         rhs=g[:, col, :],
                                         start=(done == 0),
                                         stop=(done == total - 1))
                        done += 1
                # Z window scaled by 1/deg(src)
                zw = flp.tile([128, HID], F32, tag="zw")
                if total > 0:
                    nc.scalar.activation(out=zw[:, :], in_=pw[:, :], func=AF.Copy,
                                         scale=recip_c[:, m:m + 1])
                else:
                    nc.vector.memset(zw[:, :], 0.0)
                # (Z/c) @ W2b: transpose Z window, then matmul row-major
                ptz = ps_tr.tile([128, 128], F32, tag="ptr")
                nc.tensor.transpose(out=ptz[:HID, :rows], in_=zw[:rows, :HID],
                                    identity=ident[:rows, :rows])
                zts = flp.tile([HID, 128], F32, tag="zts")
                nc.scalar.activation(out=zts[:, :rows], in_=ptz[:HID, :rows], func=AF.Copy)
                pz = ps_tr.tile([128, 128], F32, tag="pcls")
                nc.tensor.matmul(out=pz[:rows, :HID], lhsT=zts[:, :rows],
                                 rhs=W["W2b"][:, :], start=True, stop=True)
                yt4, x04, o4, n4 = yt4w[m // 4]
                co = 128 * m - o4
                xv = flp.tile([128, HID], F32, tag="xv")
                nc.vector.scalar_tensor_tensor(
                    out=xv[:rows, :], in0=tbbuf[:rows, m, :],
                    scalar=maskw[:rows, m:m + 1],
                    in1=pz[:rows, :HID], op0=ALU.mult, op1=ALU.add)
                # transpose Xv window, y = 0.5*Xv + x0h
                pt = ps_tr.tile([128, 128], F32, tag="ptr")
                nc.tensor.transpose(out=pt[:HID, :rows], in_=xv[:rows, :HID],
                                    identity=ident[:rows, :rows])
                nc.vector.scalar_tensor_tensor(
                    out=yt4[:, co:co + rows], in0=pt[:HID, :rows], scalar=0.5,
                    in1=x04[:, co:co + rows], op0=ALU.mult, op1=ALU.add)
                if m % 4 == 3 or m == c.NW - 1:
                    nc.sync.dma_start(out=yT_dram[:, o4:o4 + n4], in_=yt4[:, :n4])

            # ---------- x = relu(y @ W3 + b3) ----------
            for o, n in dense_chunks():
                yt = sb.tile([HID, CH], F32, tag="yt2")
                nc.sync.dma_start(out=yt[:, :n], in_=yT_dram[:, o:o + n])
                pd = ps_dense.tile([HID, 512], F32, tag="pd")
                nc.tensor.matmul(out=pd[:HID, :n], lhsT=W["W3"][:, :], rhs=yt[:, :n],
                                 start=True, stop=True)
                xt = sb.tile([HID, CH], F32, tag="xt4")
                nc.scalar.activation(out=xt[:, :n], in_=pd[:HID, :n], func=AF.Relu,
                                     bias=bias["b3"][:, 0:1])
                nc.sync.dma_start(out=xT_dram[:, o:o + n], in_=xt[:, :n])

        if debug_taps:
            nc.sync.dma_start(out=taps["xT"][:, :], in_=xT_dram[:, :])
            nc.sync.dma_start(out=taps["yT"][:, :], in_=yT_dram[:, :])
        # ---------- classifier + readout ----------
        gps = []
        for g in range(c.GW):
            gtile = ps_g.tile([128, c.NCLS], F32, tag=f"gps{g}")
            gps.append(gtile)
        n_mm = [0] * c.GW
        total_mm = [c.NW] * c.GW
        for o, n in dense_chunks():
            xt = sb.tile([HID, CH], F32, tag="xt5")
            nc.sync.dma_start(out=xt[:, :n], in_=xT_dram[:, o:o + n])
            pd = ps_dense.tile([HID, 512], F32, tag="pd")
            nc.tensor.matmul(out=pd[:c.CLS_H, :n], lhsT=W["Wc1"][:, :], rhs=xt[:, :n],
                             start=True, stop=True)
            ut = sb.tile([c.CLS_H, CH], F32, tag="ut2")
            nc.scalar.activation(out=ut[:, :n], in_=pd[:c.CLS_H, :n], func=AF.Relu,
                                 bias=bias["bc1"][:, 0:1])
            nb = _ceil(n, 128)
            for j in range(nb):
                b = (o + 128 * j) // 128
                r = min(128, n - 128 * j)
                pcls = ps_tr.tile([128, 128], F32, tag="pcls")
                nc.tensor.matmul(out=pcls[:r, :c.NCLS], lhsT=ut[:, 128 * j:128 * j + r],
                                 rhs=W["Wc2"][:, :], start=True, stop=True)
                cls = flp.tile([128, c.NCLS], F32, tag="cls")
                # add bc2 (replicated add via b2-style trick: bc2 [64,32]? it's [NCLS]) ->
                # bc2 is added later after readout? NO: mean of (cls+bc2) = mean(cls)+bc2.
                nc.scalar.activation(out=cls[:r, :], in_=pcls[:r, :c.NCLS], func=AF.Copy)
                for g in range(c.GW):
                    src_ids = ids_g if g == 0 else ids_g1
                    ohg = ohgp.tile([128, 128], F32, tag="ohg")
                    nc.vector.tensor_tensor(
                        out=ohg[:, :],
                        in0=src_ids[:, b:b + 1].to_broadcast([128, 128]),
                        in1=iota[:, 0, :], op=ALU.is_equal)
                    nc.tensor.matmul(out=gps[g][:, :], lhsT=ohg[:r, :],
                                     rhs=cls[:r, :],
                                     start=(n_mm[g] == 0), stop=(n_mm[g] == total_mm[g] - 1))
                    n_mm[g] += 1
        for g in range(c.GW):
            gfl = flp.tile([128, c.NCLS], F32, tag="gfl")
            nc.scalar.activation(out=gfl[:, :], in_=gps[g][:, :], func=AF.Copy)
            nc.sync.dma_start(out=gsum_part[128 * g:128 * (g + 1), :], in_=gfl[:, :])

        tc.strict_bb_all_engine_barrier()
        with tc.tile_critical():
            cc2 = nc.alloc_semaphore("cc_g")
            nc.gpsimd.collective_compute(
                "AllReduce", ALU.add, replica_groups=rg,
                ins=[gsum_part.ap().opt()], outs=[gsum_full.ap().opt()],
            ).then_inc(cc2, 1)
            nc.gpsimd.wait_ge(cc2, 1)
        tc.strict_bb_all_engine_barrier()

        # divide by counts, add bc2, write out
        for g in range(c.GW):
            gt = flp.tile([128, c.NCLS], F32, tag="gt")
            nc.sync.dma_start(out=gt[:, :], in_=gsum_full[128 * g:128 * (g + 1), :])
            go = flp.tile([128, c.NCLS], F32, tag="go")
            nc.vector.tensor_tensor(out=go[:, :], in0=gt[:, :],
                                    in1=recip_gw[:, g:g + 1].to_broadcast([128, c.NCLS]),
                                    op=ALU.mult)
            nc.vector.tensor_tensor(out=go[:, :], in0=go[:, :], in1=bc2_rep[:, :],
                                    op=ALU.add)
            rows = min(128, c.NGRAPH - 128 * g)
            nc.sync.dma_start(out=out_d[128 * g:128 * g + rows, :], in_=go[:rows, :])
        ctx.close()

    nc.finalize()
    return nc


_CACHE = {}
_LAST_RESULT = None


def _get_weights(kw, cfg):
    shapes = {
        "W_in": (cfg.FT, cfg.HID), "b_in": (cfg.HID, 1),
        "W1a": (cfg.HID, cfg.HID), "b1a": (cfg.HID, 1),
        "W1b": (cfg.HID, cfg.HID), "b1b": (cfg.HID, 1),
        "W2a": (cfg.HID, cfg.HID), "W2b": (cfg.HID, cfg.HID), "b2": (128, cfg.HID),
        "W3": (cfg.HID, cfg.HID), "b3": (cfg.HID, 1),
        "Wc1": (cfg.HID, cfg.CLS_H), "bc1": (cfg.CLS_H, 1),
        "Wc2": (cfg.CLS_H, cfg.NCLS), "bc2": (128, cfg.NCLS),
    }
    W2 = np.asarray(kw["W2"], np.float32)
    vals = {
        "W_in": kw["W_in"], "b_in": np.asarray(kw["b_in"], np.float32).reshape(-1, 1),
        "W1a": kw["W1a"], "b1a": np.asarray(kw["b1a"], np.float32).reshape(-1, 1),
        "W1b": kw["W1b"], "b1b": np.asarray(kw["b1b"], np.float32).reshape(-1, 1),
        "W2a": W2[:cfg.HID], "W2b": W2[cfg.HID:],
        "b2": np.tile(np.asarray(kw["b2"], np.float32).reshape(1, -1), (128, 1)),
        "W3": kw["W3"], "b3": np.asarray(kw["b3"], np.float32).reshape(-1, 1),
        "Wc1": kw["Wc1"], "bc1": np.asarray(kw["bc1"], np.float32).reshape(-1, 1),
        "Wc2": kw["Wc2"],
        "bc2": np.tile(np.asarray(kw["bc2"], np.float32).reshape(1, -1), (128, 1)),
    }
    vals = {k: np.ascontiguousarray(np.asarray(v, np.float32)) for k, v in vals.items()}
    return shapes, vals


def kernel(X, v2e_src, v2e_dst, all_batch, W_in, b_in, W1a, b1a, W1b, b1b,
           W2, b2, W3, b3, Wc1, bc1, Wc2, bc2, _cfg=None, _trace=False):
    cfg = _cfg or Cfg()
    kw = dict(W_in=W_in, b_in=b_in, W1a=W1a, b1a=b1a, W1b=W1b, b1b=b1b, W2=W2,
              b2=b2, W3=W3, b3=b3, Wc1=Wc1, bc1=bc1, Wc2=Wc2, bc2=bc2)
    shapes, wvals = _get_weights(kw, cfg)
    shared, in_maps = prep(cfg, np.asarray(X, np.float32), v2e_src, v2e_dst, all_batch)
    key = (cfg.N, cfg.E, tuple(shared["BA"].tolist()), tuple(shared["BL"].tolist()),
           tuple(shared["BH"].tolist()))
    if key not in _CACHE:
        _CACHE[key] = build(cfg, shared, shapes)
    nc = _CACHE[key]
    for m in in_maps:
        m.update(wvals)
    global _LAST_RESULT
    res = run_bass_kernel_spmd(nc, in_maps, core_ids=list(range(cfg.NCORES)),
                               trace=_trace)
    _LAST_RESULT = res
    return res.results[0]["out"].astype(np.float32)

